# revision 39
# baseline (speedup 1.0000x reference)
"""Fused LayerNorm + causal multi-head attention (with additive bias) + out-proj
for Trainium2, SPMD over 8 NeuronCores.

Sharding: tensor-parallel over heads. 16 heads / 8 cores = 2 heads per core.
Each core computes LN(x) (replicated), the qkv projection restricted to its
2 heads' columns, causal softmax attention with its heads' bias slices, and a
partial output projection (its heads' rows of w_out). Host sums the 8 partial
outputs (the TP all-reduce, done on gather).

Device-side layout choices:
 - x arrives pre-transposed ([dim, token], bf16) so the projections need no
   on-device transpose. LayerNorm is algebraic: with raw projections
   P = W'^T x^T (W' = gamma-scaled weights), the normalized projection is
     qkv^T[c,i] = rsig_i * (P[c,i] - u_c * mu_i) + (beta W)_c
   where u = colsum(W'). The mu and beta terms enter the same PSUM
   accumulation as rank-1 matmuls (u x (-mu*rsig) and betaW x sd), and the
   per-token rsig_i multiply rides the PSUM->SBUF eviction (one DVE
   tensor_tensor against a gpsimd-broadcast rsig row).
 - LN stats are ones-vector matmuls on the TensorEngine: mu = (1/D) 1^T x^T,
   E[x^2] = (1/D) 1^T (x^T o x^T), the square computed on GpSimd.
 - Scores are computed transposed, S^T[j, i] = (k_j . q_i), so the exp'd
   scores feed the P@V matmul directly with keys on the contraction axis.
   V is produced transposed like q/k, then turned natural by 16 PE-transposes
   per batch.
 - The causal mask and softmax max-subtraction are folded into the bias: the
   host ships bias^T pre-masked with -1e9 above the diagonal (logits here are
   O(10) so exp never overflows fp32; masked lanes hit exp(-1e9) = 0).
   Blocks entirely above the diagonal are never loaded nor computed.
 - The bias add runs on the TensorEngine as an identity-matmul accumulate
   into the scores' PSUM bank.
 - Softmax normalization is deferred: an all-ones column appended to V gives
   the row sums l_i for free; 1/l is applied to O^T after P@V.
"""

import numpy as np
import ml_dtypes
from contextlib import ExitStack

import concourse.bass as bass
import concourse.tile as tile
from concourse import bacc, mybir
from concourse.bass_utils import run_bass_kernel_spmd

F32 = mybir.dt.float32
BF16 = mybir.dt.bfloat16
AL = mybir.AluOpType

N_CORES = 8
B = 2            # batch
N = 2048         # tokens
D = 1024         # model dim
H = 16           # total heads
HL = 2           # heads per core
DH = 64          # head dim
COLS = 3 * HL * DH   # 384 qkv columns per core
KS = D // 128    # 8 contraction slabs
TT = N // 128    # 16 token tiles
IT = N // 512    # 4 i-tiles (query tiles of 512)
SCALE = DH ** -0.5
LN_EPS = 1e-5
NEG = -1.0e9

ALL_PHASES = ('prep', 'stats', 'qkv', 'attn', 'proj')


def build_program(phases=ALL_PHASES, debug=False):
    nc = bacc.Bacc("TRN2", target_bir_lowering=False, debug=False)

    xT_in = nc.dram_tensor("xT", [B, D, N], BF16, kind="ExternalInput")
    biasT_in = nc.dram_tensor("biasT", [HL, N, N], BF16, kind="ExternalInput")
    wqkv_in = nc.dram_tensor("wqkv", [D, COLS], F32, kind="ExternalInput")
    wout_in = nc.dram_tensor("wout", [HL * DH, D], F32, kind="ExternalInput")
    gamma_in = nc.dram_tensor("gamma", [D], F32, kind="ExternalInput")
    beta_in = nc.dram_tensor("beta", [D], F32, kind="ExternalInput")
    ident_in = nc.dram_tensor("ident", [128, 128], BF16, kind="ExternalInput")
    y_out = nc.dram_tensor("y", [B, N, D], F32, kind="ExternalOutput")
    if debug:
        dq = nc.dram_tensor("dq", [B, 3, 128, N], F32, kind="ExternalOutput")
        dst_dbg = nc.dram_tensor("dstat", [B, 128, N], F32, kind="ExternalOutput")

    MM = dict(skip_group_check=True)

    with tile.TileContext(nc) as tc, ExitStack() as ctx:
        # ---- persistent sbuf ----
        pers = ctx.enter_context(tc.tile_pool(name="pers", bufs=1))
        qT = [pers.tile([128, N], BF16, tag=f"qT{b}", name=f"qT{b}") for b in range(B)]
        kT = [pers.tile([128, N], BF16, tag=f"kT{b}", name=f"kT{b}") for b in range(B)]
        vT = [pers.tile([128, N], BF16, tag=f"vT{b}", name=f"vT{b}") for b in range(B)]
        # V natural with ones column: per key-tile [.., 130]: h0 v(64)+1, h1 v(64)+1
        vA = [pers.tile([128, TT, 130], BF16, tag=f"vA{b}", name=f"vA{b}") for b in range(B)]
        oT = [pers.tile([128, N], BF16, tag=f"oT{b}", name=f"oT{b}") for b in range(B)]
        ident = pers.tile([128, 128], BF16, tag="ident")
        nc.sync.dma_start(ident[:], ident_in.ap())
        onesd = pers.tile([128, 1], BF16, tag="onesd")    # 1/D for stats matmuls
        nc.vector.memset(onesd[:], 1.0 / D)
        ones1 = pers.tile([128, 1], BF16, tag="ones1")    # 1.0 for colsums
        nc.vector.memset(ones1[:], 1.0)
        epsc = pers.tile([128, 1], F32, tag="epsc")
        nc.vector.memset(epsc[:], LN_EPS)
        zeroc = pers.tile([128, 1], F32, tag="zeroc")
        nc.vector.memset(zeroc[:], 0.0)

        # ---- weights prep ----
        prep_pool = tc.tile_pool(name="prep", bufs=1)
        prep = prep_pool.__enter__()
        wq = prep.tile([128, KS, COLS], F32, tag="wq")
        nc.sync.dma_start(
            wq[:], wqkv_in.ap().rearrange("(k p) c -> p k c", p=128))
        # fold the attention scale into the q columns
        nc.vector.tensor_scalar_mul(wq[:, :, 0:128], wq[:, :, 0:128], SCALE)
        gam = prep.tile([128, KS], F32, tag="gam")
        nc.sync.dma_start(gam[:], gamma_in.ap().rearrange("(k p) -> p k", p=128))
        bet = prep.tile([128, KS], F32, tag="bet")
        nc.sync.dma_start(bet[:], beta_in.ap().rearrange("(k p) -> p k", p=128))
        wob = pers.tile([128, D], BF16, tag="wob")
        wof = prep.tile([128, D], F32, tag="wof")
        nc.sync.dma_start(wof[:], wout_in.ap())
        nc.vector.tensor_copy(wob[:], wof[:])

        # gamma-scaled bf16 qkv weights + raw bf16 (for beta @ w)
        wqb = pers.tile([128, KS, COLS], BF16, tag="wqb")
        wrb = prep.tile([128, KS, COLS], BF16, tag="wrb")
        betb = prep.tile([128, KS], BF16, tag="betb")
        nc.vector.tensor_copy(betb[:], bet[:])
        for k in range(KS):
            nc.vector.tensor_scalar_mul(wqb[:, k], wq[:, k], gam[:, k:k + 1])
            nc.vector.tensor_copy(wrb[:, k], wq[:, k])

        with tc.tile_pool(name="psmall", bufs=2, space="PSUM") as psmall:
            # bw = beta^T @ W (row form, q-cols carry SCALE via wq)
            bw_ps = psmall.tile([1, COLS], F32)
            for k in range(KS):
                nc.tensor.matmul(bw_ps[:], betb[:, k:k + 1], wrb[:, k],
                                 start=(k == 0), stop=(k == KS - 1), **MM)
            bwb = pers.tile([1, COLS], BF16, tag="bwb")
            nc.scalar.copy(bwb[:], bw_ps[:])
            # u = colsum(W') (row form), negated for the rank-1 mu correction
            u_ps = psmall.tile([1, COLS], F32)
            for k in range(KS):
                nc.tensor.matmul(u_ps[:], ones1[:], wqb[:, k],
                                 start=(k == 0), stop=(k == KS - 1), **MM)
            unb = pers.tile([1, COLS], BF16, tag="unb")
            nc.scalar.mul(unb[:], u_ps[:], -1.0)
        prep_pool.__exit__(None, None, None)

        # ---- LN stats + qkv^T, per batch ----
        rows = ctx.enter_context(tc.tile_pool(name="rows", bufs=1))
        xpool = ctx.enter_context(tc.tile_pool(name="xT", bufs=10))
        x2pool = ctx.enter_context(tc.tile_pool(name="x2", bufs=6))
        rbc = ctx.enter_context(tc.tile_pool(name="rbc", bufs=2))
        if 'stats' in phases:
            with tc.tile_pool(name="pstat", bufs=2, space="PSUM") as pstat, \
                 tc.tile_pool(name="pqkv", bufs=3, space="PSUM") as pqkv, \
                 tc.tile_pool(name="pvt", bufs=1, space="PSUM") as pvt:
                for b in range(B):
                    xTb = []
                    for k in range(KS):
                        xk = xpool.tile([128, N], BF16, tag="xk", name=f"xk{b}_{k}")
                        nc.sync.dma_start(xk[:], xT_in.ap()[b, k * 128:(k + 1) * 128, :])
                        xTb.append(xk)
                    nc.vector.memset(
                        vA[b][:, :, 64::65].rearrange("p t o -> p (t o)"), 1.0)
                    rsig_bc = rbc.tile([128, N], BF16, tag="rsig_bc", name=f"rsbc{b}")
                    s_bf = rows.tile([1, N], BF16, tag="s_bf", bufs=2,
                                     name=f"s_bf{b}")
                    sd_bf = rows.tile([1, N], BF16, tag="sd_bf", bufs=2,
                                      name=f"sd_bf{b}")
                    dsts = (qT, kT, vT)
                    # fully pipelined per 512-token slice: stats -> row chain ->
                    # qkv matmuls -> V transpose, so the PE never waits long
                    for nt in range(IT):
                        sl = slice(nt * 512, (nt + 1) * 512)
                        mu_ps = pstat.tile([1, 512], F32, tag="mu_ps")
                        for k in range(KS):
                            nc.tensor.matmul(mu_ps[:], onesd[:], xTb[k][:, sl],
                                             start=(k == 0), stop=(k == KS - 1), **MM)
                        x2_ps = pstat.tile([1, 512], F32, tag="x2_ps")
                        for k in range(KS):
                            x2 = x2pool.tile([128, 512], BF16, tag="x2")
                            nc.vector.tensor_mul(x2[:], xTb[k][:, sl], xTb[k][:, sl])
                            nc.tensor.matmul(x2_ps[:], onesd[:], x2[:],
                                             start=(k == 0), stop=(k == KS - 1), **MM)
                        # row chain on [1, 512]
                        mu_r = rows.tile([1, 512], F32, tag="mu_r", bufs=3)
                        nc.vector.tensor_copy(mu_r[:], mu_ps[:])
                        var_r = rows.tile([1, 512], F32, tag="var_r", bufs=3)
                        nc.vector.tensor_tensor(var_r[:], mu_r[:], mu_r[:], op=AL.mult)
                        nc.vector.tensor_tensor(var_r[:], x2_ps[:], var_r[:],
                                                op=AL.subtract)
                        sd_r = rows.tile([1, 512], F32, tag="sd_r", bufs=3)
                        nc.scalar.activation(sd_r[:], var_r[:],
                                             mybir.ActivationFunctionType.Sqrt,
                                             bias=epsc[0:1, :])
                        rsig_r = rows.tile([1, 512], F32, tag="rsig_r", bufs=3)
                        nc.vector.reciprocal_approx_fast(rsig_r[:], sd_r[:])
                        # rank-1 rhs is raw mu: the trailing rsig multiply
                        # supplies rsig (psum*rsig = rsig*Wx - rsig*mu*u + bw)
                        nc.vector.tensor_copy(s_bf[:, sl], mu_r[:])
                        nc.vector.tensor_copy(sd_bf[:, sl], sd_r[:])
                        rsig_bf = rows.tile([1, 512], BF16, tag="rsig_bf", bufs=3)
                        nc.vector.tensor_copy(rsig_bf[:], rsig_r[:])
                        nc.gpsimd.partition_broadcast(rsig_bc[:, sl], rsig_bf[:],
                                                      channels=128)
                        if 'qkv' in phases:
                            for blk in range(3):
                                csl = slice(blk * 128, (blk + 1) * 128)
                                ps = pqkv.tile([128, 512], F32, tag="psqkv")
                                for k in range(KS):
                                    nc.tensor.matmul(ps[:], wqb[:, k, csl],
                                                     xTb[k][:, sl],
                                                     start=(k == 0), stop=False, **MM)
                                nc.tensor.matmul(ps[:], unb[0:1, csl], s_bf[0:1, sl],
                                                 start=False, stop=False, **MM)
                                nc.tensor.matmul(ps[:], bwb[0:1, csl], sd_bf[0:1, sl],
                                                 start=False, stop=True, **MM)
                                nc.vector.tensor_tensor(dsts[blk][b][:, sl], ps[:],
                                                        rsig_bc[:, sl], op=AL.mult)
                            # V -> natural layout (ones column pre-memset)
                            for t in range(4 * nt, 4 * nt + 4):
                                pst = pvt.tile([128, 128], BF16, tag="pst")
                                nc.tensor.transpose(
                                    pst[:], vT[b][:, t * 128:(t + 1) * 128], ident[:])
                                nc.scalar.copy(
                                    vA[b][:, t, :].rearrange("p (h v) -> p h v", h=2)[:, :, 0:64],
                                    pst[:].rearrange("p (h v) -> p h v", h=2))
                    if debug and 'qkv' in phases:
                        for blk2, srcT in enumerate((qT, kT, vT)):
                            dqs = xpool.tile([128, N], F32, tag="dqs", name=f"dqs{b}_{blk2}", bufs=1)
                            nc.vector.tensor_copy(dqs[:], srcT[b][:])
                            nc.sync.dma_start(dq.ap()[b, blk2], dqs[:])
                        drs = xpool.tile([128, N], F32, tag="dqs", name=f"drs{b}", bufs=1)
                        nc.vector.tensor_copy(drs[:], rsig_bc[:])
                        nc.sync.dma_start(dst_dbg.ap()[b], drs[:])

        # ---- attention ----
        if 'attn' in phases:
            with tc.tile_pool(name="bias", bufs=16) as bias_pool, \
                 tc.tile_pool(name="pexp", bufs=10) as exp_pool, \
                 tc.tile_pool(name="lnrm", bufs=4) as lnrm, \
                 tc.tile_pool(name="pss", bufs=4, space="PSUM") as pss_pool, \
                 tc.tile_pool(name="pso", bufs=1, space="PSUM") as pso_pool:
                for t in range(IT):
                    isl = slice(t * 512, (t + 1) * 512)
                    nj = 4 * (t + 1)
                    pso = {(b, h): pso_pool.tile([65, 512], F32, tag=f"pso{b}{h}",
                                                 name=f"pso{b}{h}")
                           for b in range(B) for h in range(HL)}
                    for j in range(nj):
                        # columns i < 128j of this i-slice are fully masked:
                        # skip them in every op (causal trim)
                        off = max(0, 128 * j - 512 * t)
                        w = 512 - off
                        islo = slice(t * 512 + off, (t + 1) * 512)
                        bts = []
                        for h in range(HL):
                            bt = bias_pool.tile([128, 512], BF16, tag="bt")
                            nc.sync.dma_start(
                                bt[:, off:], biasT_in.ap()[h, j * 128:(j + 1) * 128, islo])
                            bts.append(bt)
                        use_dve = (j % 3 == 0)
                        for b in range(B):
                            pss = []
                            for h in range(HL):
                                ps = pss_pool.tile([128, 512], F32, tag="pss")
                                hsl = slice(h * 64, (h + 1) * 64)
                                nc.tensor.matmul(
                                    ps[:, off:], kT[b][hsl, j * 128:(j + 1) * 128],
                                    qT[b][hsl, islo], start=True, stop=use_dve, **MM)
                                pss.append(ps)
                            for h in range(HL):
                                if use_dve:
                                    # balance engines: bias add as in-place
                                    # PSUM RMW on the vector engine
                                    nc.vector.tensor_tensor(
                                        pss[h][:, off:], pss[h][:, off:],
                                        bts[h][:, off:], op=AL.add)
                                else:
                                    nc.tensor.matmul(pss[h][:, off:], ident[:],
                                                     bts[h][:, off:],
                                                     start=False, stop=True, **MM)
                            for h in range(HL):
                                pe = exp_pool.tile([128, 512], BF16, tag="pe")
                                nc.scalar.activation(pe[:, off:], pss[h][:, off:],
                                                     mybir.ActivationFunctionType.Exp,
                                                     bias=zeroc[:])
                                nc.tensor.matmul(
                                    pso[(b, h)][:, off:], vA[b][:, j, h * 65:h * 65 + 65],
                                    pe[:, off:], start=(j == 0), stop=(j == nj - 1), **MM)
                    for b in range(B):
                        for h in range(HL):
                            lrow = lnrm.tile([1, 512], F32, tag="lrow")
                            nc.vector.tensor_copy(lrow[:], pso[(b, h)][64:65, :])
                            rec = lnrm.tile([1, 512], F32, tag="rec")
                            nc.vector.reciprocal_approx_fast(rec[:], lrow[:])
                            lb = lnrm.tile([64, 512], F32, tag="lb")
                            nc.gpsimd.partition_broadcast(lb[:], rec[:], channels=64)
                            nc.vector.tensor_tensor(
                                oT[b][h * 64:(h + 1) * 64, isl],
                                pso[(b, h)][0:64, :], lb[:],
                                op=AL.mult)

        # ---- output projection ----
        if 'proj' in phases:
            with tc.tile_pool(name="py", bufs=2, space="PSUM") as py_pool, \
                 tc.tile_pool(name="ysb", bufs=3) as ysb:
                for b in range(B):
                    for t in range(TT):
                        psy = py_pool.tile([128, D], F32, tag="psy")
                        for half in range(2):
                            nc.tensor.matmul(psy[:, half * 512:(half + 1) * 512],
                                             oT[b][:, t * 128:(t + 1) * 128],
                                             wob[:, half * 512:(half + 1) * 512],
                                             start=True, stop=True, **MM)
                        yt = ysb.tile([128, D], F32, tag="yt")
                        if t % 2 == 0:
                            nc.vector.tensor_copy(yt[:], psy[:])
                        else:
                            nc.scalar.copy(yt[:], psy[:])
                        nc.sync.dma_start(y_out.ap()[b, t * 128:(t + 1) * 128, :], yt[:])

    nc.compile()
    return nc


_NC_CACHE = None


def _get_program():
    global _NC_CACHE
    if _NC_CACHE is None:
        _NC_CACHE = build_program()
    return _NC_CACHE


def build_in_maps(x, attn_bias, ln_gamma, ln_beta, w_qkv, w_out):
    x = np.asarray(x, dtype=np.float32)
    attn_bias = np.asarray(attn_bias, dtype=np.float32)
    ln_gamma = np.asarray(ln_gamma, dtype=np.float32)
    ln_beta = np.asarray(ln_beta, dtype=np.float32)
    w_qkv = np.asarray(w_qkv, dtype=np.float32)
    w_out = np.asarray(w_out, dtype=np.float32)

    ident = np.eye(128, dtype=ml_dtypes.bfloat16)
    xT = np.ascontiguousarray(x.transpose(0, 2, 1)).astype(ml_dtypes.bfloat16)
    # causal mask folded into the bias, transposed to [head, key j, query i]
    tri = np.triu(np.ones((N, N), dtype=bool), k=1)  # True above diag (masked)
    in_maps = []
    for c in range(N_CORES):
        h0 = HL * c
        cols = np.concatenate([
            w_qkv[:, q * H * DH + h0 * DH: q * H * DH + (h0 + HL) * DH]
            for q in range(3)], axis=1)
        biasT = np.empty((HL, N, N), dtype=ml_dtypes.bfloat16)
        for h in range(HL):
            bh = attn_bias[h0 + h].copy()
            bh[tri] = NEG
            biasT[h] = bh.T.astype(ml_dtypes.bfloat16)
        in_maps.append({
            "xT": xT,
            "biasT": biasT,
            "wqkv": np.ascontiguousarray(cols),
            "wout": np.ascontiguousarray(w_out[h0 * DH:(h0 + HL) * DH]),
            "gamma": ln_gamma,
            "beta": ln_beta,
            "ident": ident,
        })
    return in_maps


def kernel(x, attn_bias, ln_gamma, ln_beta, w_qkv, w_out):
    in_maps = build_in_maps(x, attn_bias, ln_gamma, ln_beta, w_qkv, w_out)
    nc = _get_program()
    res = run_bass_kernel_spmd(nc, in_maps, core_ids=list(range(N_CORES)))
    out = res.results[0]["y"].astype(np.float64)
    for c in range(1, N_CORES):
        out += res.results[c]["y"]
    return out.astype(np.float32)
